# revision 22
# baseline (speedup 1.0000x reference)
"""Supervised contrastive loss on 8 Trainium2 NeuronCores.

Strategy (data-parallel over embedding rows, per the sharding hint), with a
label-sorted layout so the masked work collapses to narrow windows:

  - The host sorts rows AND columns by label (the loss is permutation
    invariant). Each core owns 512 sorted rows; each 128-row m-tile's
    same-label partners then live in ONE contiguous column window of at
    most 512 columns (multinomial counts make wider windows essentially
    impossible; asserted on the host).
  - Dense path: bf16 PE matmuls compute the [128, B] similarity slab in
    PSUM chunks; one ACT Exp pass per chunk (scale=1/T) with the fused
    per-row accumulate yields sum_j exp(s_ij). The Exp output is a dead
    store - only the accumulator is consumed.
  - Window path: 4 small matmuls recompute the window's sims (bit-identical
    inputs), then exp / is_equal mask / masked-multiply / row-reduce /
    log1p run on [128, 512] tiles only (~1/8 of the columns):
      sum_same = sum_win mask * exp;  denom = sum_all - sum_same
      slog = sum_win ln(1 + me * (1/denom))   [diagonal included]
  - Per-row loss: cnt_i*ln(denom_i) + slog_i - ln(denom_i + e^{s_ii})
                  - sum_{j same} s_ij + s_ii
    where cnt, s_ii, e^{s_ii} and sum_{j same} s_ij (via class-sum matrix
    G) are exact O(B*D) host precomputes.
  - Each core writes its 512 per-row contributions; the host sums 4096
    values and divides by num_pos (exact, from label counts).
"""

import ml_dtypes
import numpy as np

import concourse.bass as bass
import concourse.bacc as bacc
import concourse.mybir as mybir
import concourse.tile as tile
from concourse.bass_utils import run_bass_kernel_spmd

B = 4096          # total rows
D = 512           # embedding dim
NCORES = 8
BL = B // NCORES  # rows per core
NK = D // 128     # contraction k-tiles
NMT = BL // 128   # output m-tiles per core
CH = 1024         # dense column chunk (2 PSUM banks)
NCH = B // CH     # dense chunks per m-tile row
WIN = 512         # same-label column window per m-tile
TINV = 10.0       # 1 / temperature
F32 = mybir.dt.float32
BF16 = mybir.dt.bfloat16
F8 = mybir.dt.float8e4
NP_F8 = mybir.dt.np(F8)
SCALE = 16.0      # fp8 pre-scale; folded out via the Exp activation scale

_CACHE = {}


def _build_nc():
    nc = bacc.Bacc()
    et = nc.dram_tensor("et", [D, B], F8, kind="ExternalInput")
    elt = nc.dram_tensor("elt", [D, BL], F8, kind="ExternalInput")
    etwin = nc.dram_tensor("etwin", [NMT, 128, NK * WIN], F8, kind="ExternalInput")
    colwin = nc.dram_tensor("colwin", [NMT, 128, WIN], BF16, kind="ExternalInput")
    meta = nc.dram_tensor("meta", [NMT, 128, 4], F32, kind="ExternalInput")
    out = nc.dram_tensor("out", [128, NMT], F32, kind="ExternalOutput")

    AF = mybir.ActivationFunctionType
    OP = mybir.AluOpType

    with tile.TileContext(nc) as tc:
        with (
            tc.tile_pool(name="const", bufs=1) as cpool,
            tc.tile_pool(name="psum", bufs=3, space=bass.MemorySpace.PSUM) as ppool,
            tc.tile_pool(name="psumw", bufs=2, space=bass.MemorySpace.PSUM) as pwpool,
            tc.tile_pool(name="chunks", bufs=3) as chpool,
            tc.tile_pool(name="winp", bufs=2) as wpool,
            tc.tile_pool(name="small", bufs=2) as smpool,
        ):
            ets = [cpool.tile([128, B], F8, tag=f"ets{k}", name=f"ets{k}")
                   for k in range(NK)]
            eltt = [cpool.tile([128, BL], F8, tag=f"elt{k}", name=f"elt{k}")
                    for k in range(NK)]
            etw_sb = [cpool.tile([128, NK * WIN], F8, tag=f"etw{m}",
                                 name=f"etw{m}") for m in range(NMT)]
            colw_sb = [cpool.tile([128, WIN], BF16, tag=f"colw{m}", name=f"colw{m}")
                       for m in range(NMT)]
            meta_sb = [cpool.tile([128, 4], F32, tag=f"meta{m}", name=f"meta{m}")
                       for m in range(NMT)]

            # Loads on the two HWDGE queues (SP + Act); gpsimd SWDGE issue
            # is ~1us/DMA and would gate the pipeline. Each ets k-tile is one
            # fully-contiguous DMA; queues alternate so transfers parallelize.
            for k in range(NK):
                eng = nc.sync if k % 2 == 0 else nc.scalar
                eng.dma_start(eltt[k][:], elt[k * 128:(k + 1) * 128, :])
                eng.dma_start(ets[k][:], et[k * 128:(k + 1) * 128, :])
            for m in range(NMT):
                eng = nc.sync if m % 2 == 0 else nc.scalar
                eng.dma_start(meta_sb[m][:], meta[m])
                eng.dma_start(colw_sb[m][:], colwin[m])
                eng.dma_start(etw_sb[m][:], etwin[m])

            sexps, mews, denoms, invs = [], [], [], []
            # ---- Phase A (Exp table set): dense accums + window pipeline --
            for mt in range(NMT):
                rowlab = meta_sb[mt][:, 0:1]
                aparts = smpool.tile([128, NCH], F32, tag="aparts")

                # dense: sum_j exp(s_ij) via fused accumulate, output dead
                for c in range(NCH):
                    psum = ppool.tile([128, CH], F32, tag="psum")
                    for k in range(NK):
                        lhsT = eltt[k][:, mt * 128:(mt + 1) * 128]
                        for h in range(CH // 512):
                            col0 = c * CH + h * 512
                            nc.tensor.matmul(
                                psum[:, h * 512:(h + 1) * 512],
                                lhsT,
                                ets[k][:, col0:col0 + 512],
                                start=(k == 0),
                                stop=(k == NK - 1),
                            )
                    dead = chpool.tile([128, CH], BF16, tag="dead")
                    nc.scalar.activation(
                        dead[:], psum[:], AF.Exp,
                        scale=TINV / (SCALE * SCALE),
                        accum_out=aparts[:, c:c + 1],
                    )

                # window: recompute the <=512 same-label columns
                psw = pwpool.tile([128, WIN], F32, tag="psw")
                for k in range(NK):
                    nc.tensor.matmul(
                        psw[:],
                        eltt[k][:, mt * 128:(mt + 1) * 128],
                        etw_sb[mt][:, k * WIN:(k + 1) * WIN],
                        start=(k == 0),
                        stop=(k == NK - 1),
                    )
                expw = wpool.tile([128, WIN], F32, tag="expw")
                last_a_act = nc.scalar.activation(
                    expw[:], psw[:], AF.Exp, scale=TINV / (SCALE * SCALE))
                maskw = wpool.tile([128, WIN], BF16, tag="maskw")
                nc.vector.tensor_scalar(
                    maskw[:], colw_sb[mt][:], rowlab, None, OP.is_equal)
                mew = wpool.tile([128, WIN], F32, tag=f"mew{mt}", name=f"mew{mt}",
                                 bufs=1)
                nc.vector.tensor_tensor(mew[:], expw[:], maskw[:], OP.mult)
                ssame = smpool.tile([128, 1], F32, tag="ssame")
                nc.vector.tensor_reduce(
                    ssame[:], mew[:], mybir.AxisListType.X, OP.add)

                sall = smpool.tile([128, 1], F32, tag="sall")
                nc.vector.tensor_reduce(
                    sall[:], aparts[:], mybir.AxisListType.X, OP.add)
                denom = smpool.tile([128, 1], F32, tag=f"denom{mt}",
                                    name=f"denom{mt}")
                nc.vector.tensor_sub(denom[:], sall[:], ssame[:])
                inv = smpool.tile([128, 1], F32, tag=f"inv{mt}", name=f"inv{mt}")
                nc.vector.reciprocal(inv[:], denom[:])
                mews.append(mew); denoms.append(denom); invs.append(inv)

            # ---- Phase B (Ln table set): all log work batched ----
            lnouts = wpool.tile([128, WIN], BF16, tag="lnout", bufs=1)
            rowtots = wpool.tile([128, NMT], F32, tag="rowtots", bufs=1)
            for mt in range(NMT):
                cnt = meta_sb[mt][:, 1:2]
                msum = meta_sb[mt][:, 2:3]
                eii = meta_sb[mt][:, 3:4]
                denom, inv, mew = denoms[mt], invs[mt], mews[mt]

                lnden = smpool.tile([128, 1], F32, tag=f"lnden{mt}",
                                    name=f"lnden{mt}")
                i_ld = nc.scalar.activation(lnden[:], denom[:], AF.Ln)
                tile.add_dep_helper(i_ld.ins, last_a_act.ins, sync=False,
                                    reason="keep Ln set after all Exp work")
                lndiag = smpool.tile([128, 1], F32, tag=f"lndiag{mt}",
                                     name=f"lndiag{mt}")
                i_lg = nc.scalar.activation(lndiag[:], eii, AF.Ln, bias=denom[:])
                tile.add_dep_helper(i_lg.ins, last_a_act.ins, sync=False,
                                    reason="keep Ln set after all Exp work")
                slog = smpool.tile([128, 1], F32, tag=f"slog{mt}",
                                   name=f"slog{mt}")
                i_sl = nc.scalar.activation(
                    lnouts[:], mew[:], AF.Ln,
                    scale=inv[:], bias=1.0, accum_out=slog[:],
                )
                tile.add_dep_helper(i_sl.ins, last_a_act.ins, sync=False,
                                    reason="keep Ln set after all Exp work")

                # rowtot = ((cnt*lnden + slog) - lndiag) + (sii - rds)
                ta = smpool.tile([128, 1], F32, tag=f"ta{mt}", name=f"ta{mt}")
                nc.vector.tensor_scalar(
                    ta[:], lnden[:], cnt, slog[:, 0:1], OP.mult, OP.add)
                nc.vector.tensor_scalar(
                    rowtots[:, mt:mt + 1], ta[:], lndiag[:, 0:1], msum,
                    OP.subtract, OP.add)
            nc.sync.dma_start(out[:], rowtots[:])
    nc.compile()
    return nc


def _make_in_maps(embeddings, labels):
    """Host-side prep: label-sort, transposes, windows, per-row scalars,
    per-core input dicts. Returns (in_maps, num_pos)."""
    emb0 = np.ascontiguousarray(np.asarray(embeddings, dtype=np.float32))
    lab0 = np.asarray(labels).astype(np.int64)
    assert emb0.shape == (B, D) and lab0.shape == (B,)

    perm = np.argsort(lab0, kind="stable")
    emb = emb0[perm]
    lab = lab0[perm]

    ET = np.ascontiguousarray(emb.T)                      # [D, B], sorted cols
    ET8 = (ET * SCALE).astype(NP_F8)
    labf = lab.astype(np.float32)
    lab16 = labf.astype(ml_dtypes.bfloat16)

    ncls = int(lab.max()) + 1
    counts = np.bincount(lab, minlength=ncls)
    cum = np.concatenate([[0], np.cumsum(counts)])
    cnt = counts[lab].astype(np.float64)                  # same-label count incl. self
    num_pos = float(cnt.sum() - B)

    emb64 = emb.astype(np.float64)
    G = np.zeros((ncls, D), np.float64)
    np.add.at(G, lab, emb64)
    rds = (emb64 * G[lab]).sum(1) * TINV                  # sum_{j same} sims_ij / T
    sii = (emb64 * emb64).sum(1) * TINV                   # sims_ii / T

    meta_all = np.stack(
        [labf.astype(np.float64), cnt, sii - rds, np.exp(sii)], axis=-1
    ).astype(np.float32)                                  # [B, 4]

    in_maps = []
    for c in range(NCORES):
        sl = slice(c * BL, (c + 1) * BL)
        etwin = np.zeros((NMT, D, WIN), NP_F8)
        colwin = np.zeros((NMT, 128, WIN), ml_dtypes.bfloat16)
        colwin[:, :, :] = ml_dtypes.bfloat16(-1.0)        # never matches a label
        for m in range(NMT):
            r0 = c * BL + m * 128
            c0 = int(cum[lab[r0]])
            c1 = int(cum[lab[r0 + 127] + 1])
            w = c1 - c0
            assert w <= WIN, f"window {w} exceeds {WIN}; rebuild with larger WIN"
            etwin[m, :, :w] = ET8[:, c0:c1]
            colwin[m, :, :w] = lab16[c0:c1][None, :]
        etwin_packed = np.ascontiguousarray(
            etwin.reshape(NMT, NK, 128, WIN).transpose(0, 2, 1, 3)
            .reshape(NMT, 128, NK * WIN))
        in_maps.append({
            "et": ET8,
            "elt": np.ascontiguousarray(ET8[:, sl]),
            "etwin": etwin_packed,
            "colwin": colwin,
            "meta": np.ascontiguousarray(meta_all[sl].reshape(NMT, 128, 4)),
        })
    return in_maps, num_pos


def kernel(embeddings, labels):
    in_maps, num_pos = _make_in_maps(embeddings, labels)
    if "nc" not in _CACHE:
        _CACHE["nc"] = _build_nc()
    nc = _CACHE["nc"]
    res = run_bass_kernel_spmd(nc, in_maps, list(range(NCORES)))
    total = sum(float(r["out"].sum()) for r in res.results)
    return np.asarray(total / max(num_pos, 1.0), dtype=np.float32)


# revision 24
# speedup vs baseline: 1.1521x; 1.1521x over previous
"""Supervised contrastive loss on 8 Trainium2 NeuronCores.

Strategy (data-parallel over embedding rows, per the sharding hint), with a
label-sorted layout so the masked work collapses to narrow windows:

  - The host sorts rows AND columns by label (the loss is permutation
    invariant). Each core owns 512 sorted rows; each 128-row m-tile's
    same-label partners then live in ONE contiguous column window of at
    most 512 columns (multinomial counts make wider windows essentially
    impossible; asserted on the host).
  - Dense path: bf16 PE matmuls compute the [128, B] similarity slab in
    PSUM chunks; one ACT Exp pass per chunk (scale=1/T) with the fused
    per-row accumulate yields sum_j exp(s_ij). The Exp output is a dead
    store - only the accumulator is consumed.
  - Window path: 4 small matmuls recompute the window's sims (bit-identical
    inputs), then exp / is_equal mask / masked-multiply / row-reduce /
    log1p run on [128, 512] tiles only (~1/8 of the columns):
      sum_same = sum_win mask * exp;  denom = sum_all - sum_same
      slog = sum_win ln(1 + me * (1/denom))   [diagonal included]
  - Per-row loss: cnt_i*ln(denom_i) + slog_i - ln(denom_i + e^{s_ii})
                  - sum_{j same} s_ij + s_ii
    where cnt, s_ii, e^{s_ii} and sum_{j same} s_ij (via class-sum matrix
    G) are exact O(B*D) host precomputes.
  - Each core writes its 512 per-row contributions; the host sums 4096
    values and divides by num_pos (exact, from label counts).
"""

import ml_dtypes
import numpy as np

import concourse.bass as bass
import concourse.bacc as bacc
import concourse.mybir as mybir
import concourse.tile as tile
from concourse.bass_utils import run_bass_kernel_spmd

B = 4096          # total rows
D = 512           # embedding dim
NCORES = 8
BL = B // NCORES  # rows per core
NK = D // 128     # contraction k-tiles
NMT = BL // 128   # output m-tiles per core
CH = 1024         # dense column chunk (2 PSUM banks)
NCH = B // CH     # dense chunks per m-tile row
WIN = 512         # same-label column window per m-tile
TINV = 10.0       # 1 / temperature
F32 = mybir.dt.float32
BF16 = mybir.dt.bfloat16
F8 = mybir.dt.float8e4
NP_F8 = mybir.dt.np(F8)
SCALE = 16.0      # fp8 pre-scale; folded out via the Exp activation scale

_CACHE = {}


def _build_nc():
    nc = bacc.Bacc()
    NKK = NK // 2     # DoubleRow k-tiles (256 contraction rows each)
    et = nc.dram_tensor("et", [NKK, 128, 2, B], F8, kind="ExternalInput")
    elt = nc.dram_tensor("elt", [NKK, 128, 2, BL], F8, kind="ExternalInput")
    etwin = nc.dram_tensor("etwin", [NMT, 128, NKK, 2, WIN], F8,
                           kind="ExternalInput")
    colwin = nc.dram_tensor("colwin", [NMT, 128, WIN], BF16, kind="ExternalInput")
    meta = nc.dram_tensor("meta", [NMT, 128, 4], F32, kind="ExternalInput")
    out = nc.dram_tensor("out", [128, NMT], F32, kind="ExternalOutput")

    AF = mybir.ActivationFunctionType
    OP = mybir.AluOpType

    with tile.TileContext(nc) as tc:
        with (
            tc.tile_pool(name="const", bufs=1) as cpool,
            tc.tile_pool(name="psum", bufs=3, space=bass.MemorySpace.PSUM) as ppool,
            tc.tile_pool(name="psumw", bufs=2, space=bass.MemorySpace.PSUM) as pwpool,
            tc.tile_pool(name="chunks", bufs=3) as chpool,
            tc.tile_pool(name="winp", bufs=2) as wpool,
            tc.tile_pool(name="small", bufs=2) as smpool,
        ):
            ets = [cpool.tile([128, 2, B], F8, tag=f"ets{k}", name=f"ets{k}")
                   for k in range(NKK)]
            eltt = [cpool.tile([128, 2, BL], F8, tag=f"elt{k}", name=f"elt{k}")
                    for k in range(NKK)]
            etw_sb = [cpool.tile([128, NKK, 2, WIN], F8, tag=f"etw{m}",
                                 name=f"etw{m}") for m in range(NMT)]
            colw_sb = [cpool.tile([128, WIN], BF16, tag=f"colw{m}", name=f"colw{m}")
                       for m in range(NMT)]
            meta_sb = [cpool.tile([128, 4], F32, tag=f"meta{m}", name=f"meta{m}")
                       for m in range(NMT)]

            # Loads on the two HWDGE queues (SP + Act); gpsimd SWDGE issue
            # is ~1us/DMA and would gate the pipeline. Each ets k-tile is one
            # fully-contiguous DMA; queues alternate so transfers parallelize.
            for k in range(NKK):
                eng = nc.sync if k % 2 == 0 else nc.scalar
                eng.dma_start(eltt[k][:], elt[k])
                eng.dma_start(ets[k][:], et[k])
            for m in range(NMT):
                eng = nc.sync if m % 2 == 0 else nc.scalar
                eng.dma_start(meta_sb[m][:], meta[m])
                eng.dma_start(colw_sb[m][:], colwin[m])
                eng.dma_start(etw_sb[m][:], etwin[m])

            sexps, mews, denoms, invs = [], [], [], []
            # ---- Phase A (Exp table set): dense accums + window pipeline --
            for mt in range(NMT):
                rowlab = meta_sb[mt][:, 0:1]
                aparts = smpool.tile([128, NCH], F32, tag="aparts")

                # dense: sum_j exp(s_ij) via fused accumulate, output dead
                for c in range(NCH):
                    psum = ppool.tile([128, CH], F32, tag="psum")
                    for k in range(NKK):
                        lhsT = eltt[k][:, :, mt * 128:(mt + 1) * 128]
                        for h in range(CH // 512):
                            col0 = c * CH + h * 512
                            nc.tensor.matmul(
                                psum[:, h * 512:(h + 1) * 512],
                                lhsT,
                                ets[k][:, :, col0:col0 + 512],
                                start=(k == 0),
                                stop=(k == NKK - 1),
                                perf_mode=mybir.MatmulPerfMode.DoubleRow,
                            )
                    dead = chpool.tile([128, CH], BF16, tag="dead")
                    nc.scalar.activation(
                        dead[:], psum[:], AF.Exp,
                        scale=TINV / (SCALE * SCALE),
                        accum_out=aparts[:, c:c + 1],
                    )

                # window: recompute the <=512 same-label columns
                psw = pwpool.tile([128, WIN], F32, tag="psw")
                for k in range(NKK):
                    nc.tensor.matmul(
                        psw[:],
                        eltt[k][:, :, mt * 128:(mt + 1) * 128],
                        etw_sb[mt][:, k],
                        start=(k == 0),
                        stop=(k == NKK - 1),
                        perf_mode=mybir.MatmulPerfMode.DoubleRow,
                    )
                expw = wpool.tile([128, WIN], F32, tag="expw")
                last_a_act = nc.scalar.activation(
                    expw[:], psw[:], AF.Exp, scale=TINV / (SCALE * SCALE))
                maskw = wpool.tile([128, WIN], BF16, tag="maskw")
                nc.vector.tensor_scalar(
                    maskw[:], colw_sb[mt][:], rowlab, None, OP.is_equal)
                mew = wpool.tile([128, WIN], F32, tag=f"mew{mt}", name=f"mew{mt}",
                                 bufs=1)
                nc.vector.tensor_tensor(mew[:], expw[:], maskw[:], OP.mult)
                ssame = smpool.tile([128, 1], F32, tag="ssame")
                nc.vector.tensor_reduce(
                    ssame[:], mew[:], mybir.AxisListType.X, OP.add)

                sall = smpool.tile([128, 1], F32, tag="sall")
                nc.vector.tensor_reduce(
                    sall[:], aparts[:], mybir.AxisListType.X, OP.add)
                denom = smpool.tile([128, 1], F32, tag=f"denom{mt}",
                                    name=f"denom{mt}")
                nc.vector.tensor_sub(denom[:], sall[:], ssame[:])
                inv = smpool.tile([128, 1], F32, tag=f"inv{mt}", name=f"inv{mt}")
                nc.vector.reciprocal(inv[:], denom[:])
                mews.append(mew); denoms.append(denom); invs.append(inv)

            # ---- Phase B (Ln table set): all log work batched ----
            lnouts = wpool.tile([128, WIN], BF16, tag="lnout", bufs=1)
            rowtots = wpool.tile([128, NMT], F32, tag="rowtots", bufs=1)
            for mt in range(NMT):
                cnt = meta_sb[mt][:, 1:2]
                msum = meta_sb[mt][:, 2:3]
                eii = meta_sb[mt][:, 3:4]
                denom, inv, mew = denoms[mt], invs[mt], mews[mt]

                lnden = smpool.tile([128, 1], F32, tag=f"lnden{mt}",
                                    name=f"lnden{mt}")
                i_ld = nc.scalar.activation(lnden[:], denom[:], AF.Ln)
                tile.add_dep_helper(i_ld.ins, last_a_act.ins, sync=False,
                                    reason="keep Ln set after all Exp work")
                lndiag = smpool.tile([128, 1], F32, tag=f"lndiag{mt}",
                                     name=f"lndiag{mt}")
                i_lg = nc.scalar.activation(lndiag[:], eii, AF.Ln, bias=denom[:])
                tile.add_dep_helper(i_lg.ins, last_a_act.ins, sync=False,
                                    reason="keep Ln set after all Exp work")
                slog = smpool.tile([128, 1], F32, tag=f"slog{mt}",
                                   name=f"slog{mt}")
                i_sl = nc.scalar.activation(
                    lnouts[:], mew[:], AF.Ln,
                    scale=inv[:], bias=1.0, accum_out=slog[:],
                )
                tile.add_dep_helper(i_sl.ins, last_a_act.ins, sync=False,
                                    reason="keep Ln set after all Exp work")

                # rowtot = ((cnt*lnden + slog) - lndiag) + (sii - rds)
                ta = smpool.tile([128, 1], F32, tag=f"ta{mt}", name=f"ta{mt}")
                nc.vector.tensor_scalar(
                    ta[:], lnden[:], cnt, slog[:, 0:1], OP.mult, OP.add)
                nc.vector.tensor_scalar(
                    rowtots[:, mt:mt + 1], ta[:], lndiag[:, 0:1], msum,
                    OP.subtract, OP.add)
            nc.sync.dma_start(out[:], rowtots[:])
    nc.compile()
    return nc


def _make_in_maps(embeddings, labels):
    """Host-side prep: label-sort, transposes, windows, per-row scalars,
    per-core input dicts. Returns (in_maps, num_pos)."""
    emb0 = np.ascontiguousarray(np.asarray(embeddings, dtype=np.float32))
    lab0 = np.asarray(labels).astype(np.int64)
    assert emb0.shape == (B, D) and lab0.shape == (B,)

    perm = np.argsort(lab0, kind="stable")
    emb = emb0[perm]
    lab = lab0[perm]

    ET = np.ascontiguousarray(emb.T)                      # [D, B], sorted cols
    ET8 = (ET * SCALE).astype(NP_F8)

    def dr_pack(a):
        # [D, X] -> [NKK, 128, 2, X] with d = kk*256 + ko*128 + ki
        X = a.shape[1]
        return np.ascontiguousarray(
            a.reshape(NK // 2, 2, 128, X).transpose(0, 2, 1, 3))
    labf = lab.astype(np.float32)
    lab16 = labf.astype(ml_dtypes.bfloat16)

    ncls = int(lab.max()) + 1
    counts = np.bincount(lab, minlength=ncls)
    cum = np.concatenate([[0], np.cumsum(counts)])
    cnt = counts[lab].astype(np.float64)                  # same-label count incl. self
    num_pos = float(cnt.sum() - B)

    emb64 = emb.astype(np.float64)
    G = np.zeros((ncls, D), np.float64)
    np.add.at(G, lab, emb64)
    rds = (emb64 * G[lab]).sum(1) * TINV                  # sum_{j same} sims_ij / T
    sii = (emb64 * emb64).sum(1) * TINV                   # sims_ii / T

    meta_all = np.stack(
        [labf.astype(np.float64), cnt, sii - rds, np.exp(sii)], axis=-1
    ).astype(np.float32)                                  # [B, 4]

    in_maps = []
    for c in range(NCORES):
        sl = slice(c * BL, (c + 1) * BL)
        etwin = np.zeros((NMT, D, WIN), NP_F8)
        colwin = np.zeros((NMT, 128, WIN), ml_dtypes.bfloat16)
        colwin[:, :, :] = ml_dtypes.bfloat16(-1.0)        # never matches a label
        for m in range(NMT):
            r0 = c * BL + m * 128
            c0 = int(cum[lab[r0]])
            c1 = int(cum[lab[r0 + 127] + 1])
            w = c1 - c0
            assert w <= WIN, f"window {w} exceeds {WIN}; rebuild with larger WIN"
            etwin[m, :, :w] = ET8[:, c0:c1]
            colwin[m, :, :w] = lab16[c0:c1][None, :]
        etwin_packed = np.ascontiguousarray(
            etwin.reshape(NMT, NK // 2, 2, 128, WIN).transpose(0, 3, 1, 2, 4))
        in_maps.append({
            "et": dr_pack(ET8),
            "elt": dr_pack(np.ascontiguousarray(ET8[:, sl])),
            "etwin": etwin_packed,
            "colwin": colwin,
            "meta": np.ascontiguousarray(meta_all[sl].reshape(NMT, 128, 4)),
        })
    return in_maps, num_pos


def kernel(embeddings, labels):
    in_maps, num_pos = _make_in_maps(embeddings, labels)
    if "nc" not in _CACHE:
        _CACHE["nc"] = _build_nc()
    nc = _CACHE["nc"]
    res = run_bass_kernel_spmd(nc, in_maps, list(range(NCORES)))
    total = sum(float(r["out"].sum()) for r in res.results)
    return np.asarray(total / max(num_pos, 1.0), dtype=np.float32)
